# revision 23
# baseline (speedup 1.0000x reference)
"""Trainium2 Bass kernel for nn_NucleicAcidFeature (retrieval_knn).

Per-graph pairwise min-atom-distance + top-k(=9) nearest neighbors + edge
construction. Sharded one graph per NeuronCore (B=8 graphs, 8 cores).

HW kernel per core (one graph, L=512 nodes, C=6 atom channels):
  - PE: 144 augmented matmuls [K=5,M=128]x[K=5,N=512] producing negated
    masked squared distances d2n[i,j;c,d] = 2*x_ic.x_jd - A_ic - A_jd,
    where A = |x|^2 + 1e30*(pad|BOS). 36 (c,d) tiles per 128-row block.
  - DVE: 36-way elementwise max chain (max of negated == min distance),
    then top-16 per row via Max8/MaxIndex/MatchReplace (k=9 <= 16).
  - Host: sqrt/clamp of 9 selected values, index offsets, sec_pos filter,
    constant global/sequential edge arrays.
"""
import os
import numpy as np

B, L, C = 8, 512, 6
N = B * L
KAUG = 5
BIG = np.float32(1e30)
BIGINT = np.float32(1e10)
NROW = L // 128  # 128-row blocks per graph

_CACHE = {}
LAST_RESULTS = None  # test.py introspection (profile/exec time)


def _build_bass():
    """Raw-Bass 4-stage pipeline per core.

    PE:  144 matmuls (36 quads of 4) into 2 alternating 4-bank PSUM quads.
    ACT: copies each PSUM quad -> one of 4 SBUF quad buffers.
    DVE: in-place 36-way max chain per 128-row block + Max8 top-16.
    SP:  input DMA + per-row-block output DMAs.

    Manual semaphores (hardware allows only one sem wait per compute
    instruction; standalone wait_ge instructions carry every cross-engine
    dependency). Sems are cleared at the tail so repeat executions of the
    loaded NEFF start clean.
    """
    import concourse.bass as bass
    import concourse.mybir as mybir
    from contextlib import ExitStack

    NQ = (C * C) // 4          # 9 quads per row-block
    TQ = NROW * NQ             # 36 quads total
    NSBQ = 4                   # SBUF quad buffers

    nc = bass.Bass("TRN2")
    uv = nc.dram_tensor("uv", [128, 2 * C * L], mybir.dt.float32,
                        kind="ExternalInput")
    # dg: [128, NROW*L + 128] = gated shifted-identity D (per row-block
    # one-hot columns scaled 2^51) followed by E = 2^50*I. The extra matmul
    # E.T @ D[:, ri*L:(ri+1)*L] accumulates 2^101*delta(i==j) into candidate
    # (0,0), forcing the (ungated) diagonal hugely positive so the host's
    # relu clamp yields dknn[i,i] == 0 exactly (matching the reference's
    # exact self-distance cancellation).
    dg = nc.dram_tensor("dg", [128, NROW * L + 128], mybir.dt.bfloat16,
                        kind="ExternalInput")
    topv = nc.dram_tensor("topv", [L, 16], mybir.dt.float32,
                          kind="ExternalOutput")
    topi = nc.dram_tensor("topi", [L, 16], mybir.dt.uint16,
                          kind="ExternalOutput")

    ctx = ExitStack()
    # uv replicated at partition offsets 0/32/64/96 so each candidate of a
    # quad runs as a concurrent row-tiled matmul (tile_position=(32t, 0)).
    uv_sb = ctx.enter_context(nc.sbuf_tensor("uv_sb", [128, 2 * C * L],
                                             mybir.dt.float32))
    dg_sb = ctx.enter_context(nc.sbuf_tensor("dg_sb", [128, NROW * L + 128],
                                             mybir.dt.bfloat16))
    sbq = [ctx.enter_context(nc.sbuf_tensor(f"sbq{i}", [128, 4, L],
                                            mybir.dt.float32))
           for i in range(NSBQ)]
    acc = ctx.enter_context(nc.sbuf_tensor("acc", [128, L],
                                           mybir.dt.float32))
    acc2 = ctx.enter_context(nc.sbuf_tensor("acc2", [128, L],
                                            mybir.dt.float32))
    acc4a = ctx.enter_context(nc.sbuf_tensor("acc4a", [128, 4, L],
                                             mybir.dt.float32))
    acc4b = ctx.enter_context(nc.sbuf_tensor("acc4b", [128, 4, L],
                                             mybir.dt.float32))
    vout = [ctx.enter_context(nc.sbuf_tensor(f"vout{i}", [128, 16],
                                             mybir.dt.float32))
            for i in range(NROW)]
    iout = [ctx.enter_context(nc.sbuf_tensor(f"iout{i}", [128, 16],
                                             mybir.dt.uint16))
            for i in range(NROW)]
    ps = [ctx.enter_context(nc.psum_tensor(f"ps{i}", [128, 4, L],
                                           mybir.dt.float32))
          for i in range(2)]
    s_dma = ctx.enter_context(nc.semaphore("s_dma"))
    s_pe = ctx.enter_context(nc.semaphore("s_pe"))
    s_act = ctx.enter_context(nc.semaphore("s_act"))
    s_dve = ctx.enter_context(nc.semaphore("s_dve"))
    s_top = ctx.enter_context(nc.semaphore("s_top"))
    s_odma = ctx.enter_context(nc.semaphore("s_odma"))
    all_sems = [s_dma, s_pe, s_act, s_dve, s_top, s_odma]

    def quad_cd(tq):
        return [(cd // C, cd % C) for cd in range(tq * 4, tq * 4 + 4)]

    with nc.Block() as block:
        CW = (2 * C * L) // 3

        @block.sync
        def _(sync):
            sync.dma_start(out=dg_sb[:, :], in_=dg[:, :]).then_inc(s_dma, 16)
            sync.dma_start(out=uv_sb[:, 0:CW],
                           in_=uv[:, 0:CW]).then_inc(s_dma, 16)
            for ri in range(NROW):
                sync.wait_ge(s_top, ri + 1)
                sync.dma_start(out=topv[ri * 128:(ri + 1) * 128, :],
                               in_=vout[ri][:, :]).then_inc(s_odma, 16)
                sync.dma_start(out=topi[ri * 128:(ri + 1) * 128, :],
                               in_=iout[ri][:, :]).then_inc(s_odma, 16)
            sync.wait_ge(s_odma, 16 * 2 * NROW)

        @block.tensor
        def _(tensor):
            tensor.wait_ge(s_dma, 16 * 4)
            for tq in range(TQ):
                ri = tq // NQ
                if tq >= 2:
                    # ACT must have drained this PSUM quad's previous use
                    tensor.wait_ge(s_act, tq - 1)
                pst = ps[tq % 2]
                for t, (c, d) in enumerate(quad_cd(tq % NQ)):
                    p0 = 32 * t
                    lhsT = uv_sb[p0:p0 + KAUG,
                                 c * L + ri * 128: c * L + ri * 128 + 128]
                    rhs = uv_sb[p0:p0 + KAUG, C * L + d * L: C * L + (d + 1) * L]
                    mm = tensor.matmul(pst[:, t, :], lhsT, rhs,
                                       start=True, stop=(tq % NQ, t) != (0, 0),
                                       tile_position=(p0, 0))
                    if (tq % NQ, t) == (0, 0):
                        # accumulate 2^101 * delta(i==j) into candidate (0,0)
                        mm = tensor.matmul(
                            pst[:, t, :],
                            dg_sb[:, NROW * L: NROW * L + 128],
                            dg_sb[:, ri * L: (ri + 1) * L],
                            start=False, stop=True)
                    if t == 3:
                        mm.then_inc(s_pe, 1)

        @block.scalar
        def _(scalar):
            scalar.dma_start(out=uv_sb[:, CW:2 * CW],
                             in_=uv[:, CW:2 * CW]).then_inc(s_dma, 16)
            scalar.dma_start(out=uv_sb[:, 2 * CW:2 * C * L],
                             in_=uv[:, 2 * CW:2 * C * L]).then_inc(s_dma, 16)
            for tq in range(TQ):
                scalar.wait_ge(s_pe, tq + 1)
                if tq >= NSBQ:
                    # DVE must have consumed this SBUF quad's previous use
                    scalar.wait_ge(s_dve, tq - NSBQ + 1)
                scalar.copy(sbq[tq % NSBQ][:, :, :],
                            ps[tq % 2][:, :, :]).then_inc(s_act, 1)

        @block.vector
        def _(vector):
            # The DVE pipeline does NOT serialize consecutive dependent ops:
            # op N+1 may read what op N is still draining (race detector
            # confirmed; intermittent corruption on HW). Keep dependent ops
            # >= 2 apart by interleaving two accumulators (acc = even
            # candidates, acc2 = odd), and drain() before each immediate
            # read-after-write in the top-k tail.
            for ri in range(NROW):
                # whole-quad (FD=2048) accumulation, two interleaved chains
                # (acc4a/acc4b) so dependent ops stay >= 2 apart; the first
                # two ops consume two freshly-copied quads each.
                t0 = ri * NQ
                vector.wait_ge(s_act, t0 + 2)
                vector.tensor_max(acc4a[:, :, :], sbq[t0 % NSBQ][:, :, :],
                                  sbq[(t0 + 1) % NSBQ][:, :, :]).then_inc(
                                      s_dve, 2)
                vector.wait_ge(s_act, t0 + 4)
                vector.tensor_max(acc4b[:, :, :], sbq[(t0 + 2) % NSBQ][:, :, :],
                                  sbq[(t0 + 3) % NSBQ][:, :, :]).then_inc(
                                      s_dve, 2)
                for q in range(4, NQ):
                    tq = t0 + q
                    vector.wait_ge(s_act, tq + 1)
                    a = acc4a if q % 2 == 0 else acc4b
                    vector.tensor_max(a[:, :, :], a[:, :, :],
                                      sbq[tq % NSBQ][:, :, :]).then_inc(
                                          s_dve, 1)
                vector.drain()
                vector.tensor_max(acc4a[:, :, :], acc4a[:, :, :],
                                  acc4b[:, :, :])
                vector.drain()
                vector.tensor_max(acc4b[:, 0:2, :], acc4a[:, 0:2, :],
                                  acc4a[:, 2:4, :])
                vector.drain()
                vector.tensor_max(acc[:, :], acc4b[:, 0, :], acc4b[:, 1, :])
                vector.drain()
                # top-16 of the row block
                vector.max(out=vout[ri][:, 0:8], in_=acc[:, :])
                vector.drain()
                vector.max_index(out=iout[ri][:, 0:8],
                                 in_max=vout[ri][:, 0:8], in_values=acc[:, :])
                vector.match_replace(out=acc2[:, :],
                                     in_to_replace=vout[ri][:, 0:8],
                                     in_values=acc[:, :], imm_value=-3e38)
                vector.drain()
                vector.max(out=vout[ri][:, 8:16], in_=acc2[:, :])
                vector.drain()
                vector.max_index(out=iout[ri][:, 8:16],
                                 in_max=vout[ri][:, 8:16],
                                 in_values=acc2[:, :]).then_inc(s_top, 1)

        @block.gpsimd
        def _(gpsimd):
            gpsimd.wait_ge(s_odma, 16 * 2 * NROW)
            for s in all_sems:
                gpsimd.sem_clear(s)

    ctx.close()
    return nc


def _get_bass():
    if "nc" not in _CACHE:
        _CACHE["nc"] = _build_bass()
    return _CACHE["nc"]


def kernel(X, AP, S, sec_pos, batch_id, k):
    global LAST_RESULTS
    from concourse.bass_utils import run_bass_kernel_spmd

    X = np.asarray(X, dtype=np.float32).reshape(N, C, 3)
    AP = np.asarray(AP).reshape(N, C)
    S = np.asarray(S).reshape(N)
    sec_pos_np = np.asarray(sec_pos).reshape(N)
    k = int(k)
    assert 1 <= k <= 16, f"kernel supports k<=16, got {k}"

    # ---- host prep: augmented vectors --------------------------------------
    # A[i,c] = x0^2 + x1^2 + x2^2 + 1e30 * (pad | BOS); masked A == 1e30 exactly
    sq = ((X[:, :, 0] * X[:, :, 0] + X[:, :, 1] * X[:, :, 1])
          + X[:, :, 2] * X[:, :, 2]).astype(np.float32)
    mask = (AP == 0) | (S == 0)[:, None]
    A = (sq + mask.astype(np.float32) * BIG).astype(np.float32)

    # u_ic = [x, A, 1] (lhs), v_jd = [2x, -1, -A] (rhs);
    # u.v = 2 x.x' - A_ic - A_jd  (= negated masked squared distance)
    ones = np.ones((N, C, 1), np.float32)
    U = np.concatenate([X, A[:, :, None], ones], axis=-1)          # [N,C,5]
    V = np.concatenate([2.0 * X, -ones, -A[:, :, None]], axis=-1)  # [N,C,5]

    # per-core uv layout [5, 2*C*L]: cols 0..C*L-1 = U (c-major, node within),
    # cols C*L.. = V. UV[kk, c*L + i] = U[i, c, kk]; UV[kk, CL + d*L + j] = V[j, d, kk]
    # dg: gated shifted identity (diag-exact-zero candidate) + 2^50*I
    gate_all = (S != 0) & ~(AP == 0).all(axis=1)   # diag boost only here
    in_maps = []
    for b in range(B):
        Ub = U[b * L:(b + 1) * L]            # [L,C,5]
        Vb = V[b * L:(b + 1) * L]
        uvb = np.zeros((128, 2 * C * L), np.float32)
        for g in range(4):
            uvb[32 * g:32 * g + KAUG, :C * L] = \
                Ub.transpose(2, 1, 0).reshape(KAUG, C * L)
            uvb[32 * g:32 * g + KAUG, C * L:] = \
                Vb.transpose(2, 1, 0).reshape(KAUG, C * L)
        dgb = np.zeros((128, NROW * L + 128), np.float32)
        gb = gate_all[b * L:(b + 1) * L]
        for ri in range(NROW):
            jj = np.arange(ri * 128, ri * 128 + 128)
            dgb[np.arange(128), ri * L + jj] = np.where(
                gb[jj], np.float32(2.0 ** 51), np.float32(0.0))
        dgb[np.arange(128), NROW * L + np.arange(128)] = np.float32(2.0 ** 50)
        import ml_dtypes
        in_maps.append({"uv": uvb, "dg": dgb.astype(ml_dtypes.bfloat16)})

    nc = _get_bass()
    res = run_bass_kernel_spmd(nc, in_maps, core_ids=list(range(B)),
                               trace=bool(os.environ.get("BASS_TRACE")))
    LAST_RESULTS = res

    topv = np.stack([res.results[b]["topv"] for b in range(B)])  # [B,L,16] f32
    topi = np.stack([res.results[b]["topi"] for b in range(B)])  # [B,L,16] u32

    # ---- host post ---------------------------------------------------------
    vals = topv.reshape(N, 16)[:, :k]                      # M = -d2 (desc)
    idx_local = topi.reshape(N, 16)[:, :k].astype(np.int64)
    m2 = np.maximum(-vals, np.float32(0.0)).astype(np.float32)
    dknn = np.minimum(np.sqrt(m2, dtype=np.float32), BIGINT).astype(np.float32)

    offs = (np.arange(B, dtype=np.int32) * L)
    dst_raw = (idx_local + np.repeat(offs, L)[:, None]).astype(np.int32)
    src = np.broadcast_to(np.arange(N, dtype=np.int32)[:, None], (N, k))
    valid = (dknn < BIGINT) & (sec_pos_np[src] == sec_pos_np[dst_raw])
    dst = np.where(valid, dst_raw, np.int32(-1)).astype(np.int32)

    # constant edge arrays (depend only on B, L)
    g_src_l = np.concatenate([np.zeros(L, np.int32),
                              np.arange(1, L, dtype=np.int32)])
    g_dst_l = np.concatenate([np.arange(L, dtype=np.int32),
                              np.zeros(L - 1, np.int32)])
    glb_edges = np.stack([
        (g_src_l[None, :] + offs[:, None]).reshape(-1),
        (g_dst_l[None, :] + offs[:, None]).reshape(-1)]).astype(np.int32)
    a = np.arange(1, L - 1, dtype=np.int32)
    s_src_l = np.concatenate([a, a + 1])
    s_dst_l = np.concatenate([a + 1, a])
    seq_edges = np.stack([
        (s_src_l[None, :] + offs[:, None]).reshape(-1),
        (s_dst_l[None, :] + offs[:, None]).reshape(-1)]).astype(np.int32)

    src = np.ascontiguousarray(src)
    return dknn, src, dst, valid, glb_edges, seq_edges
